# revision 26
# baseline (speedup 1.0000x reference)
"""DiagBlockAttention Trainium2 kernel v2 (Bass/Tile, 8 NeuronCores).

Problem (hardcoded from spec nn_DiagBlockAttention):
  x[16, 3136, 768] -> qkv = x @ w_qkv -> 12 heads x 64
  block-local attention: 56x56 token grid, 4x4 spatial blocks (16 tokens),
  softmax over the 16 tokens of each block per head
  out = attn_out @ w_out + b_out

Sharding: data-parallel over batch, 2 batches per core.

v2 design (vs v1 baseline at 983 us):
  - x is host-permuted to block order AND host-transposed to d-major
    [b, ch, 768, 448]; the output is returned d-major and un-transposed on
    the host. This deletes all 48 PE transposes per chunk (f32 transposes
    run at 2 cyc/row) and their 48 scalar-engine psum->sbuf copies.
  - PV is computed d-major: o^T[d, tq] = matmul(lhsT=v_aug[s, 65],
    rhs=pmT[s, tq]). The stationary (v_aug, with a ones column at col 64
    for the softmax sums) is ready long before the exp->mask chain, so
    LDWEIGHTS pull-ahead can hide it; v1's token-major PV used the
    freshly-masked probabilities as stationary, serializing
    LDW-wait -> MM per instruction (measured 220 ns for 27 ns of work).
  - softmax normalization: reciprocal of the sums row -> one K=2 broadcast
    matmul per head pair replicates rcp across 2x64 partitions -> one DVE
    multiply per head fuses normalization with the psum->sbuf move of o^T.
    v1 spent 260 us of DVE on 96 tiny reciprocal+scalar-mul instrs/chunk.
  - out projection d-major with N=448 moving (36 matmuls vs 48) writing
    [128, 448] psum; bias applied as per-partition tensor_scalar_add.
  - weights and x DMA directly into float32r tiles (no round-trip copies).
  - psum->sbuf casts alternate between DVE and ACT to balance engines.

Projection matmuls are float32r (full rate at free dim >= 256); attention
matmuls (scores / PV / rcp broadcast) are bf16 except the broadcast which
is f32r. Expected rel err ~2.5e-3 (same bf16 attention path as v1).
"""
import numpy as np
import ml_dtypes
from contextlib import ExitStack

import concourse.bass as bass
import concourse.mybir as mybir
import concourse.tile as tile
from concourse import bacc
from concourse.bass_utils import run_bass_kernel_spmd

# ---- problem constants ----
B, N, DIM = 16, 3136, 768
H, DH = 12, 64
J3 = 3 * H * DH              # 2304
SCALE = DH ** -0.5           # 0.125
NCORES = 8
B_LOC = B // NCORES          # 2
CHUNK = 448                  # 2 block-rows
NCHUNK = N // CHUNK          # 7
NG = 4                       # groups per chunk
GT = 112                     # tokens per group (7 blocks x 16)
KT = DIM // 128              # 6 k-tiles
F32 = mybir.dt.float32
F32R = mybir.dt.float32r
BF16 = mybir.dt.bfloat16

_CACHE = {}


def _build():
    nc = bacc.Bacc("TRN2", target_bir_lowering=False, debug=False)

    # x arrives HOST-PERMUTED to block order and TRANSPOSED d-major:
    # [b, ch, d, (g b7 ir ic)]
    x_d = nc.dram_tensor("x", [B_LOC, NCHUNK, DIM, CHUNK], BF16,
                         kind="ExternalInput")
    wqkv_d = nc.dram_tensor("w_qkv", [DIM, J3], F32, kind="ExternalInput")
    wout_d = nc.dram_tensor("w_out", [DIM, DIM], F32, kind="ExternalInput")
    bout_d = nc.dram_tensor("b_out", [DIM], F32, kind="ExternalInput")
    # output d-major in the same block order; host un-permutes
    o_d = nc.dram_tensor("o", [B_LOC, NCHUNK, DIM, CHUNK], F32,
                         kind="ExternalOutput")

    with tile.TileContext(nc) as tc, ExitStack() as ctx:
        const = ctx.enter_context(tc.tile_pool(name="const", bufs=1))
        wpool = ctx.enter_context(tc.tile_pool(name="w", bufs=1))
        xin = ctx.enter_context(tc.tile_pool(name="xin", bufs=3))
        big = ctx.enter_context(tc.tile_pool(name="big", bufs=2))
        mid = ctx.enter_context(tc.tile_pool(name="mid", bufs=6))
        nrm = ctx.enter_context(tc.tile_pool(name="nrm", bufs=3))
        outp = ctx.enter_context(tc.tile_pool(name="outp", bufs=3))

        ps_proj = ctx.enter_context(
            tc.tile_pool(name="ps_proj", bufs=2, space="PSUM"))
        ps_s = ctx.enter_context(
            tc.tile_pool(name="ps_s", bufs=3, space="PSUM"))
        ps_pv = ctx.enter_context(
            tc.tile_pool(name="ps_pv", bufs=3, space="PSUM"))

        # ---- constants ----
        # 0/1 block-diag-16 mask x4 groups: on-block iff 0 <= p - 16*b7 <= 15
        mask = const.tile([GT, NG * GT], BF16)
        nc.gpsimd.memset(mask[:], 1.0)
        mask_v = mask[:].rearrange("p (g b7 ic) -> p g b7 ic", g=NG, b7=7)
        nc.gpsimd.affine_select(
            out=mask_v, in_=mask_v, compare_op=mybir.AluOpType.is_ge,
            fill=0.0, base=0, pattern=[[0, NG], [-16, 7], [0, 16]],
            channel_multiplier=1)
        nc.gpsimd.affine_select(
            out=mask_v, in_=mask_v, compare_op=mybir.AluOpType.is_ge,
            fill=0.0, base=15, pattern=[[0, NG], [16, 7], [0, 16]],
            channel_multiplier=-1)

        # bias d-major: [128, KT] per-partition scalars
        bias_dm = const.tile([128, KT], F32)
        nc.sync.dma_start(bias_dm[:],
                          bout_d.ap().rearrange("(ko ki) -> ki ko", ki=128))

        # ---- weights: DMA f32, cast to bf16 (one-time) ----
        w_sb = wpool.tile([128, KT, J3], BF16)
        wo_sb = wpool.tile([128, KT, DIM], BF16)
        for dst, src_d, jdim in ((w_sb, wqkv_d, J3), (wo_sb, wout_d, DIM)):
            src = src_d.ap().rearrange("(ko ki) j -> ki ko j", ki=128)
            for kt in range(KT):
                for j0 in range(0, jdim, 768):
                    wtmp = mid.tile([128, 768], F32, tag="wtmp")
                    nc.sync.dma_start(wtmp[:], src[:, kt, j0:j0 + 768])
                    nc.vector.tensor_copy(dst[:, kt, j0:j0 + 768], wtmp[:])

        class OutProj:
            """Out projection of chunk (b_p, ch_p), deferred one chunk and
            emitted one dj-block at a time between attention pairs: the 6
            matmuls per block keep the PE fed while the next pair's
            exp->mask chain resolves (MATMULs execute in strict order)."""

            def __init__(self, oT_p, b_p, ch_p):
                self.oT = oT_p
                self.dst = o_d.ap()[b_p, ch_p].rearrange(
                    "(ko ki) t -> ki ko t", ki=128)
                self.out_sb = outp.tile([128, KT, CHUNK], F32, tag="out_sb")

            def emit_block(self, dj):
                op = ps_proj.tile([128, CHUNK], F32, tag="ps_proj")
                for kt in range(KT):
                    nc.tensor.matmul(
                        op[:],
                        wo_sb[:, kt, dj * 128:(dj + 1) * 128],
                        self.oT[:, kt, :],
                        start=(kt == 0), stop=(kt == KT - 1))
                nc.vector.tensor_scalar_add(self.out_sb[:, dj, :], op[:],
                                            bias_dm[:, dj:dj + 1])
                nc.sync.dma_start(self.dst[:, dj, :], self.out_sb[:, dj, :])

            def emit_all(self):
                for dj in range(KT):
                    self.emit_block(dj)

        pending = None
        for b in range(B_LOC):
            for ch in range(NCHUNK):
                # ---- A: load x^T d-major (6 split DMAs for queue spread) --
                xT = xin.tile([128, KT, CHUNK], BF16, tag="xT")
                xsrc = x_d.ap()[b, ch].rearrange("(ko ki) t -> ki ko t",
                                                 ki=128)
                for kt in range(KT):
                    nc.sync.dma_start(xT[:, kt, :], xsrc[:, kt, :])

                # ---- B: q/k projection, d-major [j-tile 128, 448] ----
                qk = big.tile([128, 12, CHUNK], BF16, tag="qk")
                for jt in range(12):
                    qkp = ps_proj.tile([128, CHUNK], F32, tag="ps_proj")
                    for kt in range(KT):
                        nc.tensor.matmul(
                            qkp[:],
                            w_sb[:, kt, jt * 128:(jt + 1) * 128],
                            xT[:, kt, :],
                            start=(kt == 0), stop=(kt == KT - 1))
                    if jt % 3 == 0:
                        nc.vector.tensor_copy(qk[:, jt, :], qkp[:])
                    else:
                        nc.scalar.copy(qk[:, jt, :], qkp[:])

                # ---- C: v projection, token-major + ones column ----
                v_sb = big.tile([GT, NG, H, 65], BF16, tag="v")
                nc.vector.memset(v_sb[:, :, :, 64], 1.0)
                for g in range(NG):
                    for half in range(2):
                        vp = ps_proj.tile([GT, 384], F32, tag="ps_proj")
                        for kt in range(KT):
                            nc.tensor.matmul(
                                vp[:],
                                xT[:, kt, g * GT:(g + 1) * GT],
                                w_sb[:, kt,
                                     1536 + half * 384:1536 + (half + 1) * 384],
                                start=(kt == 0), stop=(kt == KT - 1))
                        dst = v_sb[:, g, half * 6:(half + 1) * 6, 0:64]
                        srcv = vp[:].rearrange("p (h d) -> p h d", d=64)
                        if half == 0:
                            nc.vector.tensor_copy(dst, srcv)
                        else:
                            nc.scalar.copy(dst, srcv)

                # ---- D: attention ----
                # scores S^T[tk, (g, tq)] per head pair (even/odd heads on
                # PE row groups 0:64 / 64:128 run concurrently); exp on ACT;
                # mask on DVE; PV d-major with v_aug stationary (ready
                # early); sums row 64 -> rcp (approx, DVE) -> gpsimd
                # partition broadcast -> fused normalize+psum->sbuf multiply.
                oT = big.tile([128, KT, CHUNK], BF16, tag="oT")

                def emit_scores(hp):
                    jt_q, jt_k = hp, 6 + hp
                    sp0 = ps_s.tile([GT, NG * GT], F32, tag="ps_s",
                                    name="sp0")
                    sp1 = ps_s.tile([GT, NG * GT], F32, tag="ps_s",
                                    name="sp1")
                    for g in range(NG):
                        gs = slice(g * GT, (g + 1) * GT)
                        nc.tensor.matmul(sp0[:, gs], qk[0:64, jt_k, gs],
                                         qk[0:64, jt_q, gs],
                                         start=True, stop=True)
                        nc.tensor.matmul(sp1[:, gs], qk[64:128, jt_k, gs],
                                         qk[64:128, jt_q, gs],
                                         start=True, stop=True)
                    pm = []
                    for i, sp in enumerate((sp0, sp1)):
                        p = mid.tile([GT, NG * GT], BF16, tag="p_raw",
                                     name=f"p{i}")
                        nc.scalar.activation(p[:], sp[:],
                                             mybir.ActivationFunctionType.Exp,
                                             scale=SCALE)
                        q_ = mid.tile([GT, NG * GT], BF16, tag="p_sb",
                                      name=f"pm{i}")
                        nc.vector.tensor_mul(q_[:], p[:], mask[:])
                        pm.append(q_)
                    return pm

                def emit_pv(hp, pm):
                    # d-major PV + sums; returns (pv0, pv1, rcp)
                    pvs = []
                    sums = nrm.tile([1, 2, CHUNK], F32, tag="sums")
                    for i in range(2):
                        h = 2 * hp + i
                        pv = ps_pv.tile([65, CHUNK], F32, tag="ps_pv",
                                        name=f"pv{i}")
                        for g in range(NG):
                            gs = slice(g * GT, (g + 1) * GT)
                            nc.tensor.matmul(pv[:, gs], v_sb[:, g, h, :],
                                             pm[i][:, gs],
                                             start=True, stop=True)
                        # custom-DVE approx can't read PSUM; stage sums
                        nc.scalar.copy(sums[:, i, :], pv[64:65, :])
                        pvs.append(pv)
                    rcp = nrm.tile([1, 2, CHUNK], F32, tag="rcp")
                    nc.vector.reciprocal_approx_fast(rcp[:], sums[:])
                    return pvs[0], pvs[1], rcp

                def emit_norm(hp, pv0, pv1, rcp):
                    # replicate rcp across 64 partitions per head on the
                    # (idle) GPSIMD engine, then fuse the normalize multiply
                    # with the psum->sbuf move of o^T on DVE
                    bc_sb = nrm.tile([64, 2, CHUNK], F32, tag="bc_sb")
                    nc.gpsimd.partition_broadcast(bc_sb[:, 0, :],
                                                  rcp[:, 0, :])
                    nc.gpsimd.partition_broadcast(bc_sb[:, 1, :],
                                                  rcp[:, 1, :])
                    nc.vector.tensor_mul(oT[0:64, hp, :], pv0[0:64, :],
                                         bc_sb[:, 0, :])
                    nc.vector.tensor_mul(oT[64:128, hp, :], pv1[0:64, :],
                                         bc_sb[:, 1, :])

                pm_prev = emit_scores(0)
                pv_prev = None
                for hp in range(6):
                    if hp + 1 < 6:
                        pm_cur = emit_scores(hp + 1)
                    if pv_prev is not None:
                        emit_norm(hp - 1, *pv_prev)
                    if pending is not None:
                        pending.emit_block(hp)
                    pv_prev = emit_pv(hp, pm_prev)
                    if hp + 1 < 6:
                        pm_prev = pm_cur
                emit_norm(5, *pv_prev)
                pending = OutProj(oT, b, ch)

        pending.emit_all()

    nc.compile()
    return nc


def _to_blocks_T_host(x):
    """[B, 3136, d] raster -> bf16 [B, ch, d, (g b7 ir ic)] block order,
    d-major."""
    b, n, d = x.shape
    # n = (ch, br, ir, h2, b7, ic) with sizes (7, 2, 4, 2, 7, 4)
    x = x.reshape(b, NCHUNK, 2, 4, 2, 7, 4, d)
    # -> b ch d br h2 b7 ir ic
    x = x.transpose(0, 1, 7, 2, 4, 5, 3, 6)
    x = np.ascontiguousarray(x.reshape(b, NCHUNK, d, CHUNK))
    return x.astype(ml_dtypes.bfloat16)


def _from_blocks_T_host(o):
    """inverse of _to_blocks_T_host -> [B_sub, 3136, 768]."""
    b = o.shape[0]
    o = o.reshape(b, NCHUNK, DIM, 2, 2, 7, 4, 4)  # b ch d br h2 b7 ir ic
    o = o.transpose(0, 1, 3, 6, 4, 5, 7, 2)       # b ch br ir h2 b7 ic d
    return np.ascontiguousarray(o.reshape(b, N, DIM))


def kernel(x, w_qkv, w_out, b_out):
    x = np.ascontiguousarray(x, dtype=np.float32)
    w_qkv = np.ascontiguousarray(w_qkv, dtype=np.float32)
    w_out = np.ascontiguousarray(w_out, dtype=np.float32)
    b_out = np.ascontiguousarray(b_out, dtype=np.float32)

    if "nc" not in _CACHE:
        _CACHE["nc"] = _build()
    nc = _CACHE["nc"]

    xb = _to_blocks_T_host(x)
    in_maps = [
        {"x": xb[c * B_LOC:(c + 1) * B_LOC], "w_qkv": w_qkv,
         "w_out": w_out, "b_out": b_out}
        for c in range(NCORES)
    ]
    res = run_bass_kernel_spmd(nc, in_maps, core_ids=list(range(NCORES)))
    out = np.concatenate(
        [_from_blocks_T_host(res.results[c]["o"]) for c in range(NCORES)],
        axis=0)
    return out.astype(np.float32)


# revision 27
# speedup vs baseline: 1.0607x; 1.0607x over previous
"""DiagBlockAttention Trainium2 kernel v2 (Bass/Tile, 8 NeuronCores).

Problem (hardcoded from spec nn_DiagBlockAttention):
  x[16, 3136, 768] -> qkv = x @ w_qkv -> 12 heads x 64
  block-local attention: 56x56 token grid, 4x4 spatial blocks (16 tokens),
  softmax over the 16 tokens of each block per head
  out = attn_out @ w_out + b_out

Sharding: data-parallel over batch, 2 batches per core.

v2 design (vs v1 baseline at 983 us):
  - x is host-permuted to block order AND host-transposed to d-major
    [b, ch, 768, 448]; the output is returned d-major and un-transposed on
    the host. This deletes all 48 PE transposes per chunk (f32 transposes
    run at 2 cyc/row) and their 48 scalar-engine psum->sbuf copies.
  - PV is computed d-major: o^T[d, tq] = matmul(lhsT=v_aug[s, 65],
    rhs=pmT[s, tq]). The stationary (v_aug, with a ones column at col 64
    for the softmax sums) is ready long before the exp->mask chain, so
    LDWEIGHTS pull-ahead can hide it; v1's token-major PV used the
    freshly-masked probabilities as stationary, serializing
    LDW-wait -> MM per instruction (measured 220 ns for 27 ns of work).
  - softmax normalization: reciprocal of the sums row -> one K=2 broadcast
    matmul per head pair replicates rcp across 2x64 partitions -> one DVE
    multiply per head fuses normalization with the psum->sbuf move of o^T.
    v1 spent 260 us of DVE on 96 tiny reciprocal+scalar-mul instrs/chunk.
  - out projection d-major with N=448 moving (36 matmuls vs 48) writing
    [128, 448] psum; bias applied as per-partition tensor_scalar_add.
  - weights and x DMA directly into float32r tiles (no round-trip copies).
  - psum->sbuf casts alternate between DVE and ACT to balance engines.

Projection matmuls are float32r (full rate at free dim >= 256); attention
matmuls (scores / PV / rcp broadcast) are bf16 except the broadcast which
is f32r. Expected rel err ~2.5e-3 (same bf16 attention path as v1).
"""
import numpy as np
import ml_dtypes
from contextlib import ExitStack

import concourse.bass as bass
import concourse.mybir as mybir
import concourse.tile as tile
from concourse import bacc
from concourse.bass_utils import run_bass_kernel_spmd

# ---- problem constants ----
B, N, DIM = 16, 3136, 768
H, DH = 12, 64
J3 = 3 * H * DH              # 2304
SCALE = DH ** -0.5           # 0.125
NCORES = 8
B_LOC = B // NCORES          # 2
CHUNK = 448                  # 2 block-rows
NCHUNK = N // CHUNK          # 7
NG = 4                       # groups per chunk
GT = 112                     # tokens per group (7 blocks x 16)
KT = DIM // 128              # 6 k-tiles
F32 = mybir.dt.float32
F32R = mybir.dt.float32r
BF16 = mybir.dt.bfloat16

_CACHE = {}


def _build():
    nc = bacc.Bacc("TRN2", target_bir_lowering=False, debug=False)

    # x arrives HOST-PERMUTED to block order and TRANSPOSED d-major:
    # [b, ch, d, (g b7 ir ic)]
    x_d = nc.dram_tensor("x", [B_LOC, NCHUNK, DIM, CHUNK], BF16,
                         kind="ExternalInput")
    wqkv_d = nc.dram_tensor("w_qkv", [DIM, J3], F32, kind="ExternalInput")
    wout_d = nc.dram_tensor("w_out", [DIM, DIM], F32, kind="ExternalInput")
    bout_d = nc.dram_tensor("b_out", [DIM], F32, kind="ExternalInput")
    # output d-major in the same block order; host un-permutes
    o_d = nc.dram_tensor("o", [B_LOC, NCHUNK, DIM, CHUNK], F32,
                         kind="ExternalOutput")

    with tile.TileContext(nc) as tc, ExitStack() as ctx:
        const = ctx.enter_context(tc.tile_pool(name="const", bufs=1))
        wpool = ctx.enter_context(tc.tile_pool(name="w", bufs=1))
        xin = ctx.enter_context(tc.tile_pool(name="xin", bufs=3))
        big = ctx.enter_context(tc.tile_pool(name="big", bufs=2))
        mid = ctx.enter_context(tc.tile_pool(name="mid", bufs=4))
        nrm = ctx.enter_context(tc.tile_pool(name="nrm", bufs=2))
        outp = ctx.enter_context(tc.tile_pool(name="outp", bufs=3))

        ps_proj = ctx.enter_context(
            tc.tile_pool(name="ps_proj", bufs=2, space="PSUM"))
        ps_s = ctx.enter_context(
            tc.tile_pool(name="ps_s", bufs=3, space="PSUM"))
        ps_pv = ctx.enter_context(
            tc.tile_pool(name="ps_pv", bufs=3, space="PSUM"))

        # ---- constants ----
        # 0/1 block-diag-16 mask x4 groups: on-block iff 0 <= p - 16*b7 <= 15
        mask = const.tile([GT, NG * GT], BF16)
        nc.gpsimd.memset(mask[:], 1.0)
        mask_v = mask[:].rearrange("p (g b7 ic) -> p g b7 ic", g=NG, b7=7)
        nc.gpsimd.affine_select(
            out=mask_v, in_=mask_v, compare_op=mybir.AluOpType.is_ge,
            fill=0.0, base=0, pattern=[[0, NG], [-16, 7], [0, 16]],
            channel_multiplier=1)
        nc.gpsimd.affine_select(
            out=mask_v, in_=mask_v, compare_op=mybir.AluOpType.is_ge,
            fill=0.0, base=15, pattern=[[0, NG], [16, 7], [0, 16]],
            channel_multiplier=-1)

        # bias d-major: [128, KT] per-partition scalars
        bias_dm = const.tile([128, KT], F32)
        nc.sync.dma_start(bias_dm[:],
                          bout_d.ap().rearrange("(ko ki) -> ki ko", ki=128))

        # ---- weights: DMA f32, cast to bf16 (one-time) ----
        w_sb = wpool.tile([128, KT, J3], BF16)
        wo_sb = wpool.tile([128, KT, DIM], BF16)
        for dst, src_d, jdim in ((w_sb, wqkv_d, J3), (wo_sb, wout_d, DIM)):
            src = src_d.ap().rearrange("(ko ki) j -> ki ko j", ki=128)
            for kt in range(KT):
                for j0 in range(0, jdim, 768):
                    wtmp = mid.tile([128, 768], F32, tag="wtmp")
                    nc.sync.dma_start(wtmp[:], src[:, kt, j0:j0 + 768])
                    nc.vector.tensor_copy(dst[:, kt, j0:j0 + 768], wtmp[:])

        class OutProj:
            """Out projection of chunk (b_p, ch_p), deferred one chunk and
            emitted one dj-block at a time between attention pairs: the 6
            matmuls per block keep the PE fed while the next pair's
            exp->mask chain resolves (MATMULs execute in strict order)."""

            def __init__(self, oT_p, b_p, ch_p):
                self.oT = oT_p
                self.dst = o_d.ap()[b_p, ch_p].rearrange(
                    "(ko ki) t -> ki ko t", ki=128)
                self.out_sb = outp.tile([128, KT, CHUNK], F32, tag="out_sb")

            def emit_block(self, dj):
                op = ps_proj.tile([128, CHUNK], F32, tag="ps_proj")
                for kt in range(KT):
                    nc.tensor.matmul(
                        op[:],
                        wo_sb[:, kt, dj * 128:(dj + 1) * 128],
                        self.oT[:, kt, :],
                        start=(kt == 0), stop=(kt == KT - 1))
                nc.vector.tensor_scalar_add(self.out_sb[:, dj, :], op[:],
                                            bias_dm[:, dj:dj + 1])
                nc.sync.dma_start(self.dst[:, dj, :], self.out_sb[:, dj, :])

            def emit_all(self):
                for dj in range(KT):
                    self.emit_block(dj)

        pending = None
        for b in range(B_LOC):
            for ch in range(NCHUNK):
                # ---- A: load x^T d-major (6 split DMAs for queue spread) --
                xT = xin.tile([128, KT, CHUNK], BF16, tag="xT")
                xsrc = x_d.ap()[b, ch].rearrange("(ko ki) t -> ki ko t",
                                                 ki=128)
                for kt in range(KT):
                    nc.sync.dma_start(xT[:, kt, :], xsrc[:, kt, :])

                # ---- B: q/k projection, d-major [j-tile 128, 448] ----
                qk = big.tile([128, 12, CHUNK], BF16, tag="qk")
                for jt in range(12):
                    qkp = ps_proj.tile([128, CHUNK], F32, tag="ps_proj")
                    for kt in range(KT):
                        nc.tensor.matmul(
                            qkp[:],
                            w_sb[:, kt, jt * 128:(jt + 1) * 128],
                            xT[:, kt, :],
                            start=(kt == 0), stop=(kt == KT - 1))
                    if jt % 3 == 0:
                        nc.vector.tensor_copy(qk[:, jt, :], qkp[:])
                    else:
                        nc.scalar.copy(qk[:, jt, :], qkp[:])

                # ---- C: v projection, token-major + ones column ----
                v_sb = big.tile([GT, NG, H, 65], BF16, tag="v")
                nc.vector.memset(v_sb[:, :, :, 64], 1.0)
                for g in range(NG):
                    for half in range(2):
                        vp = ps_proj.tile([GT, 384], F32, tag="ps_proj")
                        for kt in range(KT):
                            nc.tensor.matmul(
                                vp[:],
                                xT[:, kt, g * GT:(g + 1) * GT],
                                w_sb[:, kt,
                                     1536 + half * 384:1536 + (half + 1) * 384],
                                start=(kt == 0), stop=(kt == KT - 1))
                        dst = v_sb[:, g, half * 6:(half + 1) * 6, 0:64]
                        srcv = vp[:].rearrange("p (h d) -> p h d", d=64)
                        if half == 0:
                            nc.vector.tensor_copy(dst, srcv)
                        else:
                            nc.scalar.copy(dst, srcv)

                # ---- D: attention ----
                # scores S^T[tk, (g, tq)] per head pair (even/odd heads on
                # PE row groups 0:64 / 64:128 run concurrently); exp on ACT;
                # mask on DVE; PV d-major with v_aug stationary (ready
                # early); sums row 64 -> rcp (approx, DVE) -> gpsimd
                # partition broadcast -> fused normalize+psum->sbuf multiply.
                oT = big.tile([128, KT, CHUNK], BF16, tag="oT")

                def emit_scores(hp):
                    jt_q, jt_k = hp, 6 + hp
                    sp0 = ps_s.tile([GT, NG * GT], F32, tag="ps_s",
                                    name="sp0")
                    sp1 = ps_s.tile([GT, NG * GT], F32, tag="ps_s",
                                    name="sp1")
                    for g in range(NG):
                        gs = slice(g * GT, (g + 1) * GT)
                        nc.tensor.matmul(sp0[:, gs], qk[0:64, jt_k, gs],
                                         qk[0:64, jt_q, gs],
                                         start=True, stop=True)
                        nc.tensor.matmul(sp1[:, gs], qk[64:128, jt_k, gs],
                                         qk[64:128, jt_q, gs],
                                         start=True, stop=True)
                    pm = []
                    for i, sp in enumerate((sp0, sp1)):
                        p = mid.tile([GT, NG * GT], BF16, tag="p_raw",
                                     name=f"p{i}")
                        nc.scalar.activation(p[:], sp[:],
                                             mybir.ActivationFunctionType.Exp,
                                             scale=SCALE)
                        q_ = mid.tile([GT, NG * GT], BF16, tag="p_sb",
                                      name=f"pm{i}")
                        nc.vector.tensor_mul(q_[:], p[:], mask[:])
                        pm.append(q_)
                    return pm

                def emit_pv(hp, pm):
                    # d-major PV + sums; returns (pv0, pv1, rcp)
                    pvs = []
                    sums = nrm.tile([1, 2, CHUNK], F32, tag="sums")
                    for i in range(2):
                        h = 2 * hp + i
                        pv = ps_pv.tile([65, CHUNK], F32, tag="ps_pv",
                                        name=f"pv{i}")
                        for g in range(NG):
                            gs = slice(g * GT, (g + 1) * GT)
                            nc.tensor.matmul(pv[:, gs], v_sb[:, g, h, :],
                                             pm[i][:, gs],
                                             start=True, stop=True)
                        # custom-DVE approx can't read PSUM; stage sums
                        nc.scalar.copy(sums[:, i, :], pv[64:65, :])
                        pvs.append(pv)
                    rcp = nrm.tile([1, 2, CHUNK], F32, tag="rcp")
                    nc.vector.reciprocal_approx_fast(rcp[:], sums[:])
                    return pvs[0], pvs[1], rcp

                def emit_norm(hp, pv0, pv1, rcp):
                    # replicate rcp across 64 partitions per head on the
                    # (idle) GPSIMD engine, then fuse the normalize multiply
                    # with the psum->sbuf move of o^T on DVE
                    bc_sb = nrm.tile([64, 2, CHUNK], F32, tag="bc_sb")
                    nc.gpsimd.partition_broadcast(bc_sb[:, 0, :],
                                                  rcp[:, 0, :])
                    nc.gpsimd.partition_broadcast(bc_sb[:, 1, :],
                                                  rcp[:, 1, :])
                    nc.vector.tensor_mul(oT[0:64, hp, :], pv0[0:64, :],
                                         bc_sb[:, 0, :])
                    nc.vector.tensor_mul(oT[64:128, hp, :], pv1[0:64, :],
                                         bc_sb[:, 1, :])

                pm_prev = emit_scores(0)
                pv_prev = None
                for hp in range(6):
                    if hp + 1 < 6:
                        pm_cur = emit_scores(hp + 1)
                    if pv_prev is not None:
                        emit_norm(hp - 1, *pv_prev)
                    if pending is not None:
                        pending.emit_block(hp)
                    pv_prev = emit_pv(hp, pm_prev)
                    if hp + 1 < 6:
                        pm_prev = pm_cur
                emit_norm(5, *pv_prev)
                pending = OutProj(oT, b, ch)

        pending.emit_all()

    nc.compile()
    return nc


def _to_blocks_T_host(x):
    """[B, 3136, d] raster -> bf16 [B, ch, d, (g b7 ir ic)] block order,
    d-major."""
    b, n, d = x.shape
    # n = (ch, br, ir, h2, b7, ic) with sizes (7, 2, 4, 2, 7, 4)
    x = x.reshape(b, NCHUNK, 2, 4, 2, 7, 4, d)
    # -> b ch d br h2 b7 ir ic
    x = x.transpose(0, 1, 7, 2, 4, 5, 3, 6)
    x = np.ascontiguousarray(x.reshape(b, NCHUNK, d, CHUNK))
    return x.astype(ml_dtypes.bfloat16)


def _from_blocks_T_host(o):
    """inverse of _to_blocks_T_host -> [B_sub, 3136, 768]."""
    b = o.shape[0]
    o = o.reshape(b, NCHUNK, DIM, 2, 2, 7, 4, 4)  # b ch d br h2 b7 ir ic
    o = o.transpose(0, 1, 3, 6, 4, 5, 7, 2)       # b ch br ir h2 b7 ic d
    return np.ascontiguousarray(o.reshape(b, N, DIM))


def kernel(x, w_qkv, w_out, b_out):
    x = np.ascontiguousarray(x, dtype=np.float32)
    w_qkv = np.ascontiguousarray(w_qkv, dtype=np.float32)
    w_out = np.ascontiguousarray(w_out, dtype=np.float32)
    b_out = np.ascontiguousarray(b_out, dtype=np.float32)

    if "nc" not in _CACHE:
        _CACHE["nc"] = _build()
    nc = _CACHE["nc"]

    xb = _to_blocks_T_host(x)
    in_maps = [
        {"x": xb[c * B_LOC:(c + 1) * B_LOC], "w_qkv": w_qkv,
         "w_out": w_out, "b_out": b_out}
        for c in range(NCORES)
    ]
    res = run_bass_kernel_spmd(nc, in_maps, core_ids=list(range(NCORES)))
    out = np.concatenate(
        [_from_blocks_T_host(res.results[c]["o"]) for c in range(NCORES)],
        axis=0)
    return out.astype(np.float32)


# revision 28
# speedup vs baseline: 1.0666x; 1.0056x over previous
"""DiagBlockAttention Trainium2 kernel v2 (Bass/Tile, 8 NeuronCores).

Problem (hardcoded from spec nn_DiagBlockAttention):
  x[16, 3136, 768] -> qkv = x @ w_qkv -> 12 heads x 64
  block-local attention: 56x56 token grid, 4x4 spatial blocks (16 tokens),
  softmax over the 16 tokens of each block per head
  out = attn_out @ w_out + b_out

Sharding: data-parallel over batch, 2 batches per core.

v2 design (652 us vs the 983 us v1 baseline; rel err 4.3e-3 < 2e-2):
  - x is host-permuted to block order AND host-transposed to d-major bf16
    [b, ch, 768, 448]; the output is returned d-major and un-permuted on
    the host. Deletes all 48 PE transposes per chunk (f32 transposes run
    at 2 cyc/row) and their 48 scalar-engine psum->sbuf copies.
  - everything on the PE is bf16 (psum accumulation stays f32): enables
    FWL weight loads, halves x DMA and SBUF footprints. Host-simulated
    all-bf16 numerics give 4.3e-3 max rel err vs the f32 reference.
  - PV is computed d-major: o^T[d, tq] = matmul(lhsT=v_aug[s, 65],
    rhs=pmT[s, tq]). The stationary (v_aug, ones column at col 64 yields
    the softmax sums as psum row 64) is ready long before the exp->mask
    chain; v1's token-major PV used the freshly-masked probabilities as
    stationary, serializing LDW-wait -> MM (220 ns for 27 ns of work).
  - softmax normalization: sums row staged to SBUF (ACT), batched
    reciprocal_approx_fast per pair (DVE custom op, ~5x cheaper than
    RECIPROCAL and psum-capable ops can't do it), gpsimd
    partition_broadcast onto 64 partitions (idle Pool engine), then one
    DVE multiply per head fuses normalization with the psum->sbuf move
    of o^T. v1 burned 260 us of DVE on 96 tiny rcp+scalar-mul instrs.
  - out projection d-major with N=448 moving, DEFERRED one chunk and
    interleaved one dj-block per attention pair: those 6 matmuls cover
    exactly the exp->mask dependency window (MATMULs execute in strict
    program order, so a stalled matmul blocks the whole PE queue).
  - psum->sbuf casts split between DVE and ACT to balance queues.

Measured (per-core trace at 652 us): PE busy 81% (529 us, within ~8%% of
its streaming floor), DVE 57%, ACT 47%, Pool 23%.
"""
import numpy as np
import ml_dtypes
from contextlib import ExitStack

import concourse.bass as bass
import concourse.mybir as mybir
import concourse.tile as tile
from concourse import bacc
from concourse.bass_utils import run_bass_kernel_spmd

# ---- problem constants ----
B, N, DIM = 16, 3136, 768
H, DH = 12, 64
J3 = 3 * H * DH              # 2304
SCALE = DH ** -0.5           # 0.125
NCORES = 8
B_LOC = B // NCORES          # 2
CHUNK = 448                  # 2 block-rows
NCHUNK = N // CHUNK          # 7
NG = 4                       # groups per chunk
GT = 112                     # tokens per group (7 blocks x 16)
KT = DIM // 128              # 6 k-tiles
F32 = mybir.dt.float32
F32R = mybir.dt.float32r
BF16 = mybir.dt.bfloat16

_CACHE = {}


def _build():
    nc = bacc.Bacc("TRN2", target_bir_lowering=False, debug=False)

    # x arrives HOST-PERMUTED to block order and TRANSPOSED d-major:
    # [b, ch, d, (g b7 ir ic)]
    x_d = nc.dram_tensor("x", [B_LOC, NCHUNK, DIM, CHUNK], BF16,
                         kind="ExternalInput")
    wqkv_d = nc.dram_tensor("w_qkv", [DIM, J3], F32, kind="ExternalInput")
    wout_d = nc.dram_tensor("w_out", [DIM, DIM], F32, kind="ExternalInput")
    bout_d = nc.dram_tensor("b_out", [DIM], F32, kind="ExternalInput")
    # output d-major in the same block order; host un-permutes
    o_d = nc.dram_tensor("o", [B_LOC, NCHUNK, DIM, CHUNK], F32,
                         kind="ExternalOutput")

    with tile.TileContext(nc) as tc, ExitStack() as ctx:
        const = ctx.enter_context(tc.tile_pool(name="const", bufs=1))
        wpool = ctx.enter_context(tc.tile_pool(name="w", bufs=1))
        xin = ctx.enter_context(tc.tile_pool(name="xin", bufs=3))
        big = ctx.enter_context(tc.tile_pool(name="big", bufs=2))
        mid = ctx.enter_context(tc.tile_pool(name="mid", bufs=4))
        nrm = ctx.enter_context(tc.tile_pool(name="nrm", bufs=2))
        outp = ctx.enter_context(tc.tile_pool(name="outp", bufs=3))

        ps_proj = ctx.enter_context(
            tc.tile_pool(name="ps_proj", bufs=2, space="PSUM"))
        ps_s = ctx.enter_context(
            tc.tile_pool(name="ps_s", bufs=3, space="PSUM"))
        ps_pv = ctx.enter_context(
            tc.tile_pool(name="ps_pv", bufs=3, space="PSUM"))

        # ---- constants ----
        # 0/1 block-diag-16 mask x4 groups: on-block iff 0 <= p - 16*b7 <= 15
        mask = const.tile([GT, NG * GT], BF16)
        nc.gpsimd.memset(mask[:], 1.0)
        mask_v = mask[:].rearrange("p (g b7 ic) -> p g b7 ic", g=NG, b7=7)
        nc.gpsimd.affine_select(
            out=mask_v, in_=mask_v, compare_op=mybir.AluOpType.is_ge,
            fill=0.0, base=0, pattern=[[0, NG], [-16, 7], [0, 16]],
            channel_multiplier=1)
        nc.gpsimd.affine_select(
            out=mask_v, in_=mask_v, compare_op=mybir.AluOpType.is_ge,
            fill=0.0, base=15, pattern=[[0, NG], [16, 7], [0, 16]],
            channel_multiplier=-1)

        # bias d-major: [128, KT] per-partition scalars
        bias_dm = const.tile([128, KT], F32)
        nc.sync.dma_start(bias_dm[:],
                          bout_d.ap().rearrange("(ko ki) -> ki ko", ki=128))

        # ---- weights: DMA f32, cast to bf16 (one-time) ----
        w_sb = wpool.tile([128, KT, J3], BF16)
        wo_sb = wpool.tile([128, KT, DIM], BF16)
        for dst, src_d, jdim in ((w_sb, wqkv_d, J3), (wo_sb, wout_d, DIM)):
            src = src_d.ap().rearrange("(ko ki) j -> ki ko j", ki=128)
            for kt in range(KT):
                for j0 in range(0, jdim, 768):
                    wtmp = mid.tile([128, 768], F32, tag="wtmp")
                    nc.sync.dma_start(wtmp[:], src[:, kt, j0:j0 + 768])
                    nc.vector.tensor_copy(dst[:, kt, j0:j0 + 768], wtmp[:])

        class OutProj:
            """Out projection of chunk (b_p, ch_p), deferred one chunk and
            emitted one dj-block at a time between attention pairs: the 6
            matmuls per block keep the PE fed while the next pair's
            exp->mask chain resolves (MATMULs execute in strict order)."""

            def __init__(self, oT_p, b_p, ch_p):
                self.oT = oT_p
                self.dst = o_d.ap()[b_p, ch_p].rearrange(
                    "(ko ki) t -> ki ko t", ki=128)
                self.out_sb = outp.tile([128, KT, CHUNK], F32, tag="out_sb")

            def emit_block(self, dj):
                op = ps_proj.tile([128, CHUNK], F32, tag="ps_proj")
                for kt in range(KT):
                    nc.tensor.matmul(
                        op[:],
                        wo_sb[:, kt, dj * 128:(dj + 1) * 128],
                        self.oT[:, kt, :],
                        start=(kt == 0), stop=(kt == KT - 1))
                nc.vector.tensor_scalar_add(self.out_sb[:, dj, :], op[:],
                                            bias_dm[:, dj:dj + 1])
                nc.sync.dma_start(self.dst[:, dj, :], self.out_sb[:, dj, :])

            def emit_all(self):
                for dj in range(KT):
                    self.emit_block(dj)

        pending = None
        for b in range(B_LOC):
            for ch in range(NCHUNK):
                # ---- A: load x^T d-major (6 split DMAs for queue spread) --
                xT = xin.tile([128, KT, CHUNK], BF16, tag="xT")
                xsrc = x_d.ap()[b, ch].rearrange("(ko ki) t -> ki ko t",
                                                 ki=128)
                for kt in range(KT):
                    nc.sync.dma_start(xT[:, kt, :], xsrc[:, kt, :])

                # ---- B: q/k projection, d-major [j-tile 128, 448] ----
                qk = big.tile([128, 12, CHUNK], BF16, tag="qk")
                for jt in range(12):
                    qkp = ps_proj.tile([128, CHUNK], F32, tag="ps_proj")
                    for kt in range(KT):
                        nc.tensor.matmul(
                            qkp[:],
                            w_sb[:, kt, jt * 128:(jt + 1) * 128],
                            xT[:, kt, :],
                            start=(kt == 0), stop=(kt == KT - 1))
                    if jt % 3 == 0:
                        nc.vector.tensor_copy(qk[:, jt, :], qkp[:])
                    else:
                        nc.scalar.copy(qk[:, jt, :], qkp[:])

                # ---- C: v projection, token-major + ones column ----
                v_sb = big.tile([GT, NG, H, 65], BF16, tag="v")
                nc.vector.memset(v_sb[:, :, :, 64], 1.0)
                for g in range(NG):
                    for half in range(2):
                        vp = ps_proj.tile([GT, 384], F32, tag="ps_proj")
                        for kt in range(KT):
                            nc.tensor.matmul(
                                vp[:],
                                xT[:, kt, g * GT:(g + 1) * GT],
                                w_sb[:, kt,
                                     1536 + half * 384:1536 + (half + 1) * 384],
                                start=(kt == 0), stop=(kt == KT - 1))
                        dst = v_sb[:, g, half * 6:(half + 1) * 6, 0:64]
                        srcv = vp[:].rearrange("p (h d) -> p h d", d=64)
                        if half == 0:
                            nc.vector.tensor_copy(dst, srcv)
                        else:
                            nc.scalar.copy(dst, srcv)

                # ---- D: attention ----
                # scores S^T[tk, (g, tq)] per head pair (even/odd heads on
                # PE row groups 0:64 / 64:128 run concurrently); exp on ACT;
                # mask on DVE; PV d-major with v_aug stationary (ready
                # early); sums row 64 -> rcp (approx, DVE) -> gpsimd
                # partition broadcast -> fused normalize+psum->sbuf multiply.
                oT = big.tile([128, KT, CHUNK], BF16, tag="oT")

                def emit_scores(hp):
                    jt_q, jt_k = hp, 6 + hp
                    sp0 = ps_s.tile([GT, NG * GT], F32, tag="ps_s",
                                    name="sp0")
                    sp1 = ps_s.tile([GT, NG * GT], F32, tag="ps_s",
                                    name="sp1")
                    for g in range(NG):
                        gs = slice(g * GT, (g + 1) * GT)
                        nc.tensor.matmul(sp0[:, gs], qk[0:64, jt_k, gs],
                                         qk[0:64, jt_q, gs],
                                         start=True, stop=True)
                        nc.tensor.matmul(sp1[:, gs], qk[64:128, jt_k, gs],
                                         qk[64:128, jt_q, gs],
                                         start=True, stop=True)
                    pm = []
                    for i, sp in enumerate((sp0, sp1)):
                        p = mid.tile([GT, NG * GT], BF16, tag="p_raw",
                                     name=f"p{i}")
                        nc.scalar.activation(p[:], sp[:],
                                             mybir.ActivationFunctionType.Exp,
                                             scale=SCALE)
                        q_ = mid.tile([GT, NG * GT], BF16, tag="p_sb",
                                      name=f"pm{i}")
                        nc.vector.tensor_mul(q_[:], p[:], mask[:])
                        pm.append(q_)
                    return pm

                def emit_pv(hp, pm):
                    # d-major PV + sums; returns (pv0, pv1, rcp)
                    pvs = []
                    sums = nrm.tile([1, 2, CHUNK], F32, tag="sums")
                    for i in range(2):
                        h = 2 * hp + i
                        pv = ps_pv.tile([65, CHUNK], F32, tag="ps_pv",
                                        name=f"pv{i}")
                        for g in range(NG):
                            gs = slice(g * GT, (g + 1) * GT)
                            nc.tensor.matmul(pv[:, gs], v_sb[:, g, h, :],
                                             pm[i][:, gs],
                                             start=True, stop=True)
                        # custom-DVE approx can't read PSUM; stage sums
                        nc.scalar.copy(sums[:, i, :], pv[64:65, :])
                        pvs.append(pv)
                    rcp = nrm.tile([1, 2, CHUNK], F32, tag="rcp")
                    nc.vector.reciprocal_approx_fast(rcp[:], sums[:])
                    return pvs[0], pvs[1], rcp

                def emit_norm(hp, pv0, pv1, rcp):
                    # replicate rcp across 64 partitions per head on the
                    # (idle) GPSIMD engine, then fuse the normalize multiply
                    # with the psum->sbuf move of o^T on DVE
                    bc_sb = nrm.tile([64, 2, CHUNK], F32, tag="bc_sb")
                    nc.gpsimd.partition_broadcast(bc_sb[:, 0, :],
                                                  rcp[:, 0, :])
                    nc.gpsimd.partition_broadcast(bc_sb[:, 1, :],
                                                  rcp[:, 1, :])
                    nc.vector.tensor_mul(oT[0:64, hp, :], pv0[0:64, :],
                                         bc_sb[:, 0, :])
                    nc.vector.tensor_mul(oT[64:128, hp, :], pv1[0:64, :],
                                         bc_sb[:, 1, :])

                pm_prev = emit_scores(0)
                pv_prev = None
                for hp in range(6):
                    if hp + 1 < 6:
                        pm_cur = emit_scores(hp + 1)
                    if pv_prev is not None:
                        emit_norm(hp - 1, *pv_prev)
                    if pending is not None:
                        pending.emit_block(hp)
                    pv_prev = emit_pv(hp, pm_prev)
                    if hp + 1 < 6:
                        pm_prev = pm_cur
                emit_norm(5, *pv_prev)
                pending = OutProj(oT, b, ch)

        pending.emit_all()

    nc.compile()
    return nc


def _to_blocks_T_host(x):
    """[B, 3136, d] raster -> bf16 [B, ch, d, (g b7 ir ic)] block order,
    d-major."""
    b, n, d = x.shape
    # n = (ch, br, ir, h2, b7, ic) with sizes (7, 2, 4, 2, 7, 4)
    x = x.reshape(b, NCHUNK, 2, 4, 2, 7, 4, d)
    # -> b ch d br h2 b7 ir ic
    x = x.transpose(0, 1, 7, 2, 4, 5, 3, 6)
    x = np.ascontiguousarray(x.reshape(b, NCHUNK, d, CHUNK))
    return x.astype(ml_dtypes.bfloat16)


def _from_blocks_T_host(o):
    """inverse of _to_blocks_T_host -> [B_sub, 3136, 768]."""
    b = o.shape[0]
    o = o.reshape(b, NCHUNK, DIM, 2, 2, 7, 4, 4)  # b ch d br h2 b7 ir ic
    o = o.transpose(0, 1, 3, 6, 4, 5, 7, 2)       # b ch br ir h2 b7 ic d
    return np.ascontiguousarray(o.reshape(b, N, DIM))


def kernel(x, w_qkv, w_out, b_out):
    x = np.ascontiguousarray(x, dtype=np.float32)
    w_qkv = np.ascontiguousarray(w_qkv, dtype=np.float32)
    w_out = np.ascontiguousarray(w_out, dtype=np.float32)
    b_out = np.ascontiguousarray(b_out, dtype=np.float32)

    if "nc" not in _CACHE:
        _CACHE["nc"] = _build()
    nc = _CACHE["nc"]

    xb = _to_blocks_T_host(x)
    in_maps = [
        {"x": xb[c * B_LOC:(c + 1) * B_LOC], "w_qkv": w_qkv,
         "w_out": w_out, "b_out": b_out}
        for c in range(NCORES)
    ]
    res = run_bass_kernel_spmd(nc, in_maps, core_ids=list(range(NCORES)))
    out = np.concatenate(
        [_from_blocks_T_host(res.results[c]["o"]) for c in range(NCORES)],
        axis=0)
    return out.astype(np.float32)


# revision 29
# speedup vs baseline: 1.1390x; 1.0678x over previous
"""DiagBlockAttention Trainium2 kernel v2 (Bass/Tile, 8 NeuronCores).

Problem (hardcoded from spec nn_DiagBlockAttention):
  x[16, 3136, 768] -> qkv = x @ w_qkv -> 12 heads x 64
  block-local attention: 56x56 token grid, 4x4 spatial blocks (16 tokens),
  softmax over the 16 tokens of each block per head
  out = attn_out @ w_out + b_out

Sharding: data-parallel over batch, 2 batches per core.

v2 design (652 us vs the 983 us v1 baseline; rel err 4.3e-3 < 2e-2):
  - x is host-permuted to block order AND host-transposed to d-major bf16
    [b, ch, 768, 448]; the output is returned d-major and un-permuted on
    the host. Deletes all 48 PE transposes per chunk (f32 transposes run
    at 2 cyc/row) and their 48 scalar-engine psum->sbuf copies.
  - everything on the PE is bf16 (psum accumulation stays f32): enables
    FWL weight loads, halves x DMA and SBUF footprints. Host-simulated
    all-bf16 numerics give 4.3e-3 max rel err vs the f32 reference.
  - PV is computed d-major: o^T[d, tq] = matmul(lhsT=v_aug[s, 65],
    rhs=pmT[s, tq]). The stationary (v_aug, ones column at col 64 yields
    the softmax sums as psum row 64) is ready long before the exp->mask
    chain; v1's token-major PV used the freshly-masked probabilities as
    stationary, serializing LDW-wait -> MM (220 ns for 27 ns of work).
  - softmax normalization: sums row staged to SBUF (ACT), batched
    reciprocal_approx_fast per pair (DVE custom op, ~5x cheaper than
    RECIPROCAL and psum-capable ops can't do it), gpsimd
    partition_broadcast onto 64 partitions (idle Pool engine), then one
    DVE multiply per head fuses normalization with the psum->sbuf move
    of o^T. v1 burned 260 us of DVE on 96 tiny rcp+scalar-mul instrs.
  - out projection d-major with N=448 moving, DEFERRED one chunk and
    interleaved one dj-block per attention pair: those 6 matmuls cover
    exactly the exp->mask dependency window (MATMULs execute in strict
    program order, so a stalled matmul blocks the whole PE queue).
  - psum->sbuf casts split between DVE and ACT to balance queues.

Measured (per-core trace at 652 us): PE busy 81% (529 us, within ~8%% of
its streaming floor), DVE 57%, ACT 47%, Pool 23%.
"""
import numpy as np
import ml_dtypes
from contextlib import ExitStack

import concourse.bass as bass
import concourse.mybir as mybir
import concourse.tile as tile
from concourse import bacc
from concourse.bass_utils import run_bass_kernel_spmd

# ---- problem constants ----
B, N, DIM = 16, 3136, 768
H, DH = 12, 64
J3 = 3 * H * DH              # 2304
SCALE = DH ** -0.5           # 0.125
NCORES = 8
B_LOC = B // NCORES          # 2
CHUNK = 448                  # 2 block-rows
NCHUNK = N // CHUNK          # 7
NG = 4                       # groups per chunk
GT = 112                     # tokens per group (7 blocks x 16)
KT = DIM // 128              # 6 k-tiles
F32 = mybir.dt.float32
F32R = mybir.dt.float32r
BF16 = mybir.dt.bfloat16

_CACHE = {}


def _build():
    nc = bacc.Bacc("TRN2", target_bir_lowering=False, debug=False)

    # x arrives HOST-PERMUTED to block order and TRANSPOSED d-major:
    # [b, ch, d, (g b7 ir ic)]
    x_d = nc.dram_tensor("x", [B_LOC, NCHUNK, DIM, CHUNK], BF16,
                         kind="ExternalInput")
    wqkv_d = nc.dram_tensor("w_qkv", [DIM, J3], F32, kind="ExternalInput")
    wout_d = nc.dram_tensor("w_out", [DIM, DIM], F32, kind="ExternalInput")
    bout_d = nc.dram_tensor("b_out", [DIM], F32, kind="ExternalInput")
    # output d-major in the same block order; host un-permutes
    o_d = nc.dram_tensor("o", [B_LOC, NCHUNK, DIM, CHUNK], F32,
                         kind="ExternalOutput")

    with tile.TileContext(nc) as tc, ExitStack() as ctx:
        const = ctx.enter_context(tc.tile_pool(name="const", bufs=1))
        wpool = ctx.enter_context(tc.tile_pool(name="w", bufs=1))
        xin = ctx.enter_context(tc.tile_pool(name="xin", bufs=3))
        big = ctx.enter_context(tc.tile_pool(name="big", bufs=2))
        mid = ctx.enter_context(tc.tile_pool(name="mid", bufs=4))
        nrm = ctx.enter_context(tc.tile_pool(name="nrm", bufs=2))
        outp = ctx.enter_context(tc.tile_pool(name="outp", bufs=3))

        ps_proj = ctx.enter_context(
            tc.tile_pool(name="ps_proj", bufs=3, space="PSUM"))
        ps_s = ctx.enter_context(
            tc.tile_pool(name="ps_s", bufs=3, space="PSUM"))
        ps_pv = ctx.enter_context(
            tc.tile_pool(name="ps_pv", bufs=2, space="PSUM"))

        # ---- constants ----
        # 0/1 block-diag-16 mask x4 groups: on-block iff 0 <= p - 16*b7 <= 15
        mask = const.tile([GT, NG * GT], BF16)
        nc.gpsimd.memset(mask[:], 1.0)
        mask_v = mask[:].rearrange("p (g b7 ic) -> p g b7 ic", g=NG, b7=7)
        nc.gpsimd.affine_select(
            out=mask_v, in_=mask_v, compare_op=mybir.AluOpType.is_ge,
            fill=0.0, base=0, pattern=[[0, NG], [-16, 7], [0, 16]],
            channel_multiplier=1)
        nc.gpsimd.affine_select(
            out=mask_v, in_=mask_v, compare_op=mybir.AluOpType.is_ge,
            fill=0.0, base=15, pattern=[[0, NG], [16, 7], [0, 16]],
            channel_multiplier=-1)

        # bias d-major: [128, KT] per-partition scalars
        bias_dm = const.tile([128, KT], F32)
        nc.sync.dma_start(bias_dm[:],
                          bout_d.ap().rearrange("(ko ki) -> ki ko", ki=128))

        # ---- weights: DMA f32, cast to bf16 (one-time) ----
        w_sb = wpool.tile([128, KT, J3], BF16)
        wo_sb = wpool.tile([128, KT, DIM], BF16)
        for dst, src_d, jdim in ((w_sb, wqkv_d, J3), (wo_sb, wout_d, DIM)):
            src = src_d.ap().rearrange("(ko ki) j -> ki ko j", ki=128)
            for kt in range(KT):
                for j0 in range(0, jdim, 768):
                    wtmp = mid.tile([128, 768], F32, tag="wtmp")
                    nc.sync.dma_start(wtmp[:], src[:, kt, j0:j0 + 768])
                    nc.vector.tensor_copy(dst[:, kt, j0:j0 + 768], wtmp[:])

        class OutProj:
            """Out projection of chunk (b_p, ch_p), deferred one chunk and
            emitted one dj-block at a time between attention pairs: the 6
            matmuls per block keep the PE fed while the next pair's
            exp->mask chain resolves (MATMULs execute in strict order)."""

            def __init__(self, oT_p, b_p, ch_p):
                self.oT = oT_p
                self.dst = o_d.ap()[b_p, ch_p].rearrange(
                    "(ko ki) t -> ki ko t", ki=128)
                self.out_sb = outp.tile([128, KT, CHUNK], F32, tag="out_sb")

            def emit_block(self, dj):
                op = ps_proj.tile([128, CHUNK], F32, tag="ps_proj")
                for kt in range(KT):
                    nc.tensor.matmul(
                        op[:],
                        wo_sb[:, kt, dj * 128:(dj + 1) * 128],
                        self.oT[:, kt, :],
                        start=(kt == 0), stop=(kt == KT - 1))
                nc.vector.tensor_scalar_add(self.out_sb[:, dj, :], op[:],
                                            bias_dm[:, dj:dj + 1])
                nc.sync.dma_start(self.dst[:, dj, :], self.out_sb[:, dj, :])

            def emit_all(self):
                for dj in range(KT):
                    self.emit_block(dj)

        pending = None
        for b in range(B_LOC):
            for ch in range(NCHUNK):
                # ---- A: load x^T d-major (6 split DMAs for queue spread) --
                xT = xin.tile([128, KT, CHUNK], BF16, tag="xT")
                xsrc = x_d.ap()[b, ch].rearrange("(ko ki) t -> ki ko t",
                                                 ki=128)
                for kt in range(KT):
                    nc.sync.dma_start(xT[:, kt, :], xsrc[:, kt, :])

                # ---- B: q/k projection, d-major [j-tile 128, 448] ----
                qk = big.tile([128, 12, CHUNK], BF16, tag="qk")
                for jt in range(12):
                    qkp = ps_proj.tile([128, CHUNK], F32, tag="ps_proj")
                    for kt in range(KT):
                        nc.tensor.matmul(
                            qkp[:],
                            w_sb[:, kt, jt * 128:(jt + 1) * 128],
                            xT[:, kt, :],
                            start=(kt == 0), stop=(kt == KT - 1))
                    if jt % 3 == 0:
                        nc.vector.tensor_copy(qk[:, jt, :], qkp[:])
                    else:
                        nc.scalar.copy(qk[:, jt, :], qkp[:])

                # ---- C: v projection, token-major + ones column ----
                v_sb = big.tile([GT, NG, H, 65], BF16, tag="v")
                nc.vector.memset(v_sb[:, :, :, 64], 1.0)
                for g in range(NG):
                    for half in range(2):
                        vp = ps_proj.tile([GT, 384], F32, tag="ps_proj")
                        for kt in range(KT):
                            nc.tensor.matmul(
                                vp[:],
                                xT[:, kt, g * GT:(g + 1) * GT],
                                w_sb[:, kt,
                                     1536 + half * 384:1536 + (half + 1) * 384],
                                start=(kt == 0), stop=(kt == KT - 1))
                        dst = v_sb[:, g, half * 6:(half + 1) * 6, 0:64]
                        srcv = vp[:].rearrange("p (h d) -> p h d", d=64)
                        if half == 0:
                            nc.vector.tensor_copy(dst, srcv)
                        else:
                            nc.scalar.copy(dst, srcv)

                # ---- D: attention ----
                # scores S^T[tk, (g, tq)] per head pair (even/odd heads on
                # PE row groups 0:64 / 64:128 run concurrently); exp on ACT;
                # mask on DVE; PV d-major with v_aug stationary (ready
                # early); sums row 64 -> rcp (approx, DVE) -> gpsimd
                # partition broadcast -> fused normalize+psum->sbuf multiply.
                oT = big.tile([128, KT, CHUNK], BF16, tag="oT")

                def emit_scores(hp):
                    jt_q, jt_k = hp, 6 + hp
                    sp0 = ps_s.tile([GT, NG * GT], F32, tag="ps_s",
                                    name="sp0")
                    sp1 = ps_s.tile([GT, NG * GT], F32, tag="ps_s",
                                    name="sp1")
                    for g in range(NG):
                        gs = slice(g * GT, (g + 1) * GT)
                        nc.tensor.matmul(sp0[:, gs], qk[0:64, jt_k, gs],
                                         qk[0:64, jt_q, gs],
                                         start=True, stop=True)
                        nc.tensor.matmul(sp1[:, gs], qk[64:128, jt_k, gs],
                                         qk[64:128, jt_q, gs],
                                         start=True, stop=True)
                    pm = []
                    for i, sp in enumerate((sp0, sp1)):
                        p = mid.tile([GT, NG * GT], BF16, tag="p_raw",
                                     name=f"p{i}")
                        nc.scalar.activation(p[:], sp[:],
                                             mybir.ActivationFunctionType.Exp,
                                             scale=SCALE)
                        q_ = mid.tile([GT, NG * GT], BF16, tag="p_sb",
                                      name=f"pm{i}")
                        nc.vector.tensor_mul(q_[:], p[:], mask[:])
                        pm.append(q_)
                    return pm

                def emit_pv(hp, pm):
                    # d-major PV + sums; returns (pv0, pv1, rcp)
                    pvs = []
                    sums = nrm.tile([1, 2, CHUNK], F32, tag="sums")
                    for i in range(2):
                        h = 2 * hp + i
                        pv = ps_pv.tile([65, CHUNK], F32, tag="ps_pv",
                                        name=f"pv{i}")
                        for g in range(NG):
                            gs = slice(g * GT, (g + 1) * GT)
                            nc.tensor.matmul(pv[:, gs], v_sb[:, g, h, :],
                                             pm[i][:, gs],
                                             start=True, stop=True)
                        # custom-DVE approx can't read PSUM; stage sums
                        nc.scalar.copy(sums[:, i, :], pv[64:65, :])
                        pvs.append(pv)
                    rcp = nrm.tile([1, 2, CHUNK], F32, tag="rcp")
                    nc.vector.reciprocal_approx_fast(rcp[:], sums[:])
                    return pvs[0], pvs[1], rcp

                def emit_norm(hp, pv0, pv1, rcp):
                    # replicate rcp across 64 partitions per head on the
                    # (idle) GPSIMD engine, then fuse the normalize multiply
                    # with the psum->sbuf move of o^T on DVE
                    bc_sb = nrm.tile([64, 2, CHUNK], F32, tag="bc_sb")
                    nc.gpsimd.partition_broadcast(bc_sb[:, 0, :],
                                                  rcp[:, 0, :])
                    nc.gpsimd.partition_broadcast(bc_sb[:, 1, :],
                                                  rcp[:, 1, :])
                    nc.vector.tensor_mul(oT[0:64, hp, :], pv0[0:64, :],
                                         bc_sb[:, 0, :])
                    nc.vector.tensor_mul(oT[64:128, hp, :], pv1[0:64, :],
                                         bc_sb[:, 1, :])

                pm_prev = emit_scores(0)
                pv_prev = None
                for hp in range(6):
                    if hp + 1 < 6:
                        pm_cur = emit_scores(hp + 1)
                    if pv_prev is not None:
                        emit_norm(hp - 1, *pv_prev)
                    if pending is not None:
                        pending.emit_block(hp)
                    pv_prev = emit_pv(hp, pm_prev)
                    if hp + 1 < 6:
                        pm_prev = pm_cur
                emit_norm(5, *pv_prev)
                pending = OutProj(oT, b, ch)

        pending.emit_all()

    nc.compile()
    return nc


def _to_blocks_T_host(x):
    """[B, 3136, d] raster -> bf16 [B, ch, d, (g b7 ir ic)] block order,
    d-major."""
    b, n, d = x.shape
    # n = (ch, br, ir, h2, b7, ic) with sizes (7, 2, 4, 2, 7, 4)
    x = x.reshape(b, NCHUNK, 2, 4, 2, 7, 4, d)
    # -> b ch d br h2 b7 ir ic
    x = x.transpose(0, 1, 7, 2, 4, 5, 3, 6)
    x = np.ascontiguousarray(x.reshape(b, NCHUNK, d, CHUNK))
    return x.astype(ml_dtypes.bfloat16)


def _from_blocks_T_host(o):
    """inverse of _to_blocks_T_host -> [B_sub, 3136, 768]."""
    b = o.shape[0]
    o = o.reshape(b, NCHUNK, DIM, 2, 2, 7, 4, 4)  # b ch d br h2 b7 ir ic
    o = o.transpose(0, 1, 3, 6, 4, 5, 7, 2)       # b ch br ir h2 b7 ic d
    return np.ascontiguousarray(o.reshape(b, N, DIM))


def kernel(x, w_qkv, w_out, b_out):
    x = np.ascontiguousarray(x, dtype=np.float32)
    w_qkv = np.ascontiguousarray(w_qkv, dtype=np.float32)
    w_out = np.ascontiguousarray(w_out, dtype=np.float32)
    b_out = np.ascontiguousarray(b_out, dtype=np.float32)

    if "nc" not in _CACHE:
        _CACHE["nc"] = _build()
    nc = _CACHE["nc"]

    xb = _to_blocks_T_host(x)
    in_maps = [
        {"x": xb[c * B_LOC:(c + 1) * B_LOC], "w_qkv": w_qkv,
         "w_out": w_out, "b_out": b_out}
        for c in range(NCORES)
    ]
    res = run_bass_kernel_spmd(nc, in_maps, core_ids=list(range(NCORES)))
    out = np.concatenate(
        [_from_blocks_T_host(res.results[c]["o"]) for c in range(NCORES)],
        axis=0)
    return out.astype(np.float32)


# revision 30
# speedup vs baseline: 1.1546x; 1.0137x over previous
"""DiagBlockAttention Trainium2 kernel v2 (Bass/Tile, 8 NeuronCores).

Problem (hardcoded from spec nn_DiagBlockAttention):
  x[16, 3136, 768] -> qkv = x @ w_qkv -> 12 heads x 64
  block-local attention: 56x56 token grid, 4x4 spatial blocks (16 tokens),
  softmax over the 16 tokens of each block per head
  out = attn_out @ w_out + b_out

Sharding: data-parallel over batch, 2 batches per core.

v2 design (652 us vs the 983 us v1 baseline; rel err 4.3e-3 < 2e-2):
  - x is host-permuted to block order AND host-transposed to d-major bf16
    [b, ch, 768, 448]; the output is returned d-major and un-permuted on
    the host. Deletes all 48 PE transposes per chunk (f32 transposes run
    at 2 cyc/row) and their 48 scalar-engine psum->sbuf copies.
  - everything on the PE is bf16 (psum accumulation stays f32): enables
    FWL weight loads, halves x DMA and SBUF footprints. Host-simulated
    all-bf16 numerics give 4.3e-3 max rel err vs the f32 reference.
  - PV is computed d-major: o^T[d, tq] = matmul(lhsT=v_aug[s, 65],
    rhs=pmT[s, tq]). The stationary (v_aug, ones column at col 64 yields
    the softmax sums as psum row 64) is ready long before the exp->mask
    chain; v1's token-major PV used the freshly-masked probabilities as
    stationary, serializing LDW-wait -> MM (220 ns for 27 ns of work).
  - softmax normalization: sums row staged to SBUF (ACT), batched
    reciprocal_approx_fast per pair (DVE custom op, ~5x cheaper than
    RECIPROCAL and psum-capable ops can't do it), gpsimd
    partition_broadcast onto 64 partitions (idle Pool engine), then one
    DVE multiply per head fuses normalization with the psum->sbuf move
    of o^T. v1 burned 260 us of DVE on 96 tiny rcp+scalar-mul instrs.
  - out projection d-major with N=448 moving, DEFERRED one chunk and
    interleaved one dj-block per attention pair: those 6 matmuls cover
    exactly the exp->mask dependency window (MATMULs execute in strict
    program order, so a stalled matmul blocks the whole PE queue).
  - psum->sbuf casts split between DVE and ACT to balance queues.

Measured (per-core trace at 652 us): PE busy 81% (529 us, within ~8%% of
its streaming floor), DVE 57%, ACT 47%, Pool 23%.
"""
import numpy as np
import ml_dtypes
from contextlib import ExitStack

import concourse.bass as bass
import concourse.mybir as mybir
import concourse.tile as tile
from concourse import bacc
from concourse.bass_utils import run_bass_kernel_spmd

# ---- problem constants ----
B, N, DIM = 16, 3136, 768
H, DH = 12, 64
J3 = 3 * H * DH              # 2304
SCALE = DH ** -0.5           # 0.125
NCORES = 8
B_LOC = B // NCORES          # 2
CHUNK = 448                  # 2 block-rows
NCHUNK = N // CHUNK          # 7
NG = 4                       # groups per chunk
GT = 112                     # tokens per group (7 blocks x 16)
KT = DIM // 128              # 6 k-tiles
F32 = mybir.dt.float32
F32R = mybir.dt.float32r
BF16 = mybir.dt.bfloat16

_CACHE = {}


def _build():
    nc = bacc.Bacc("TRN2", target_bir_lowering=False, debug=False)

    # x arrives HOST-PERMUTED to block order and TRANSPOSED d-major:
    # [b, ch, d, (g b7 ir ic)]
    x_d = nc.dram_tensor("x", [B_LOC, NCHUNK, DIM, CHUNK], BF16,
                         kind="ExternalInput")
    wqkv_d = nc.dram_tensor("w_qkv", [DIM, J3], F32, kind="ExternalInput")
    wout_d = nc.dram_tensor("w_out", [DIM, DIM], F32, kind="ExternalInput")
    bout_d = nc.dram_tensor("b_out", [DIM], F32, kind="ExternalInput")
    # output d-major in the same block order; host un-permutes
    o_d = nc.dram_tensor("o", [B_LOC, NCHUNK, DIM, CHUNK], F32,
                         kind="ExternalOutput")

    with tile.TileContext(nc) as tc, ExitStack() as ctx:
        const = ctx.enter_context(tc.tile_pool(name="const", bufs=1))
        wpool = ctx.enter_context(tc.tile_pool(name="w", bufs=1))
        xin = ctx.enter_context(tc.tile_pool(name="xin", bufs=3))
        big = ctx.enter_context(tc.tile_pool(name="big", bufs=2))
        mid = ctx.enter_context(tc.tile_pool(name="mid", bufs=4))
        nrm = ctx.enter_context(tc.tile_pool(name="nrm", bufs=2))
        outp = ctx.enter_context(tc.tile_pool(name="outp", bufs=3))

        ps_proj = ctx.enter_context(
            tc.tile_pool(name="ps_proj", bufs=3, space="PSUM"))
        ps_s = ctx.enter_context(
            tc.tile_pool(name="ps_s", bufs=3, space="PSUM"))
        ps_pv = ctx.enter_context(
            tc.tile_pool(name="ps_pv", bufs=2, space="PSUM"))

        # ---- constants ----
        # 0/1 block-diag-16 mask x4 groups: on-block iff 0 <= p - 16*b7 <= 15
        mask = const.tile([GT, NG * GT], BF16)
        nc.gpsimd.memset(mask[:], 1.0)
        mask_v = mask[:].rearrange("p (g b7 ic) -> p g b7 ic", g=NG, b7=7)
        nc.gpsimd.affine_select(
            out=mask_v, in_=mask_v, compare_op=mybir.AluOpType.is_ge,
            fill=0.0, base=0, pattern=[[0, NG], [-16, 7], [0, 16]],
            channel_multiplier=1)
        nc.gpsimd.affine_select(
            out=mask_v, in_=mask_v, compare_op=mybir.AluOpType.is_ge,
            fill=0.0, base=15, pattern=[[0, NG], [16, 7], [0, 16]],
            channel_multiplier=-1)

        # bias d-major: [128, KT] per-partition scalars
        bias_dm = const.tile([128, KT], F32)
        nc.sync.dma_start(bias_dm[:],
                          bout_d.ap().rearrange("(ko ki) -> ki ko", ki=128))

        # ---- weights: DMA f32, cast to bf16 (one-time) ----
        w_sb = wpool.tile([128, KT, J3], BF16)
        wo_sb = wpool.tile([128, KT, DIM], BF16)
        for dst, src_d, jdim in ((w_sb, wqkv_d, J3), (wo_sb, wout_d, DIM)):
            src = src_d.ap().rearrange("(ko ki) j -> ki ko j", ki=128)
            for kt in range(KT):
                for j0 in range(0, jdim, 768):
                    wtmp = mid.tile([128, 768], F32, tag="wtmp")
                    nc.sync.dma_start(wtmp[:], src[:, kt, j0:j0 + 768])
                    nc.vector.tensor_copy(dst[:, kt, j0:j0 + 768], wtmp[:])

        class OutProj:
            """Out projection of chunk (b_p, ch_p), deferred one chunk and
            emitted one dj-block at a time between attention pairs: the 6
            matmuls per block keep the PE fed while the next pair's
            exp->mask chain resolves (MATMULs execute in strict order)."""

            def __init__(self, oT_p, b_p, ch_p):
                self.oT = oT_p
                self.dst = o_d.ap()[b_p, ch_p].rearrange(
                    "(ko ki) t -> ki ko t", ki=128)
                self.out_sb = outp.tile([128, KT, CHUNK], F32, tag="out_sb")

            def emit_block(self, dj):
                op = ps_proj.tile([128, CHUNK], F32, tag="ps_proj")
                for kt in range(KT):
                    nc.tensor.matmul(
                        op[:],
                        wo_sb[:, kt, dj * 128:(dj + 1) * 128],
                        self.oT[:, kt, :],
                        start=(kt == 0), stop=(kt == KT - 1))
                nc.vector.tensor_scalar_add(self.out_sb[:, dj, :], op[:],
                                            bias_dm[:, dj:dj + 1])
                nc.sync.dma_start(self.dst[:, dj, :], self.out_sb[:, dj, :])

            def emit_all(self):
                for dj in range(KT):
                    self.emit_block(dj)

        pending = None
        for b in range(B_LOC):
            for ch in range(NCHUNK):
                # ---- A: load x^T d-major (6 split DMAs for queue spread) --
                xT = xin.tile([128, KT, CHUNK], BF16, tag="xT")
                xsrc = x_d.ap()[b, ch].rearrange("(ko ki) t -> ki ko t",
                                                 ki=128)
                for kt in range(KT):
                    nc.sync.dma_start(xT[:, kt, :], xsrc[:, kt, :])

                # ---- B: q/k projection, d-major [j-tile 128, 448] ----
                # padded to 464 cols so score stationaries are [64, 128]
                # (triggers FWL); garbage cols feed psum rows 112:128 which
                # are never read
                qk = big.tile([128, 12, CHUNK + 16], BF16, tag="qk")
                for jt in range(12):
                    qkp = ps_proj.tile([128, CHUNK], F32, tag="ps_proj")
                    for kt in range(KT):
                        nc.tensor.matmul(
                            qkp[:],
                            w_sb[:, kt, jt * 128:(jt + 1) * 128],
                            xT[:, kt, :],
                            start=(kt == 0), stop=(kt == KT - 1))
                    if jt % 3 == 0:
                        nc.vector.tensor_copy(qk[:, jt, 0:CHUNK], qkp[:])
                    else:
                        nc.scalar.copy(qk[:, jt, 0:CHUNK], qkp[:])

                # ---- C: v projection, token-major + ones column ----
                v_sb = big.tile([GT, NG, H, 65], BF16, tag="v")
                nc.vector.memset(v_sb[:, :, :, 64], 1.0)
                for g in range(NG):
                    for half in range(2):
                        vp = ps_proj.tile([GT, 384], F32, tag="ps_proj")
                        for kt in range(KT):
                            nc.tensor.matmul(
                                vp[:],
                                xT[:, kt, g * GT:(g + 1) * GT],
                                w_sb[:, kt,
                                     1536 + half * 384:1536 + (half + 1) * 384],
                                start=(kt == 0), stop=(kt == KT - 1))
                        dst = v_sb[:, g, half * 6:(half + 1) * 6, 0:64]
                        srcv = vp[:].rearrange("p (h d) -> p h d", d=64)
                        if half == 0:
                            nc.vector.tensor_copy(dst, srcv)
                        else:
                            nc.scalar.copy(dst, srcv)

                # ---- D: attention ----
                # scores S^T[tk, (g, tq)] per head pair (even/odd heads on
                # PE row groups 0:64 / 64:128 run concurrently); exp on ACT;
                # mask on DVE; PV d-major with v_aug stationary (ready
                # early); sums row 64 -> rcp (approx, DVE) -> gpsimd
                # partition broadcast -> fused normalize+psum->sbuf multiply.
                oT = big.tile([128, KT, CHUNK], BF16, tag="oT")

                def emit_scores(hp):
                    jt_q, jt_k = hp, 6 + hp
                    sp0 = ps_s.tile([128, NG * GT], F32, tag="ps_s",
                                    name="sp0")
                    sp1 = ps_s.tile([128, NG * GT], F32, tag="ps_s",
                                    name="sp1")
                    for g in range(NG):
                        gs = slice(g * GT, (g + 1) * GT)
                        gk = slice(g * GT, g * GT + 128)
                        nc.tensor.matmul(sp0[:, gs], qk[0:64, jt_k, gk],
                                         qk[0:64, jt_q, gs],
                                         start=True, stop=True)
                        nc.tensor.matmul(sp1[:, gs], qk[64:128, jt_k, gk],
                                         qk[64:128, jt_q, gs],
                                         start=True, stop=True)
                    pm = []
                    for i, sp in enumerate((sp0, sp1)):
                        p = mid.tile([GT, NG * GT], BF16, tag="p_raw",
                                     name=f"p{i}")
                        nc.scalar.activation(p[:], sp[0:GT, :],
                                             mybir.ActivationFunctionType.Exp,
                                             scale=SCALE)
                        q_ = mid.tile([GT, NG * GT], BF16, tag="p_sb",
                                      name=f"pm{i}")
                        nc.vector.tensor_mul(q_[:], p[:], mask[:])
                        pm.append(q_)
                    return pm

                def emit_pv(hp, pm):
                    # d-major PV + sums; returns (pv0, pv1, rcp)
                    pvs = []
                    sums = nrm.tile([1, 2, CHUNK], F32, tag="sums")
                    for i in range(2):
                        h = 2 * hp + i
                        pv = ps_pv.tile([65, CHUNK], F32, tag="ps_pv",
                                        name=f"pv{i}")
                        for g in range(NG):
                            gs = slice(g * GT, (g + 1) * GT)
                            nc.tensor.matmul(pv[:, gs], v_sb[:, g, h, :],
                                             pm[i][:, gs],
                                             start=True, stop=True)
                        # custom-DVE approx can't read PSUM; stage sums
                        nc.scalar.copy(sums[:, i, :], pv[64:65, :])
                        pvs.append(pv)
                    rcp = nrm.tile([1, 2, CHUNK], F32, tag="rcp")
                    nc.vector.reciprocal_approx_fast(rcp[:], sums[:])
                    return pvs[0], pvs[1], rcp

                def emit_norm(hp, pv0, pv1, rcp):
                    # replicate rcp across 64 partitions per head on the
                    # (idle) GPSIMD engine, then fuse the normalize multiply
                    # with the psum->sbuf move of o^T on DVE
                    bc_sb = nrm.tile([64, 2, CHUNK], F32, tag="bc_sb")
                    nc.gpsimd.partition_broadcast(bc_sb[:, 0, :],
                                                  rcp[:, 0, :])
                    nc.gpsimd.partition_broadcast(bc_sb[:, 1, :],
                                                  rcp[:, 1, :])
                    nc.vector.tensor_mul(oT[0:64, hp, :], pv0[0:64, :],
                                         bc_sb[:, 0, :])
                    nc.vector.tensor_mul(oT[64:128, hp, :], pv1[0:64, :],
                                         bc_sb[:, 1, :])

                pm_prev = emit_scores(0)
                pv_prev = None
                for hp in range(6):
                    if hp + 1 < 6:
                        pm_cur = emit_scores(hp + 1)
                    if pv_prev is not None:
                        emit_norm(hp - 1, *pv_prev)
                    if pending is not None:
                        pending.emit_block(hp)
                    pv_prev = emit_pv(hp, pm_prev)
                    if hp + 1 < 6:
                        pm_prev = pm_cur
                emit_norm(5, *pv_prev)
                pending = OutProj(oT, b, ch)

        pending.emit_all()

    nc.compile()
    return nc


def _to_blocks_T_host(x):
    """[B, 3136, d] raster -> bf16 [B, ch, d, (g b7 ir ic)] block order,
    d-major."""
    b, n, d = x.shape
    # n = (ch, br, ir, h2, b7, ic) with sizes (7, 2, 4, 2, 7, 4)
    x = x.reshape(b, NCHUNK, 2, 4, 2, 7, 4, d)
    # -> b ch d br h2 b7 ir ic
    x = x.transpose(0, 1, 7, 2, 4, 5, 3, 6)
    x = np.ascontiguousarray(x.reshape(b, NCHUNK, d, CHUNK))
    return x.astype(ml_dtypes.bfloat16)


def _from_blocks_T_host(o):
    """inverse of _to_blocks_T_host -> [B_sub, 3136, 768]."""
    b = o.shape[0]
    o = o.reshape(b, NCHUNK, DIM, 2, 2, 7, 4, 4)  # b ch d br h2 b7 ir ic
    o = o.transpose(0, 1, 3, 6, 4, 5, 7, 2)       # b ch br ir h2 b7 ic d
    return np.ascontiguousarray(o.reshape(b, N, DIM))


def kernel(x, w_qkv, w_out, b_out):
    x = np.ascontiguousarray(x, dtype=np.float32)
    w_qkv = np.ascontiguousarray(w_qkv, dtype=np.float32)
    w_out = np.ascontiguousarray(w_out, dtype=np.float32)
    b_out = np.ascontiguousarray(b_out, dtype=np.float32)

    if "nc" not in _CACHE:
        _CACHE["nc"] = _build()
    nc = _CACHE["nc"]

    xb = _to_blocks_T_host(x)
    in_maps = [
        {"x": xb[c * B_LOC:(c + 1) * B_LOC], "w_qkv": w_qkv,
         "w_out": w_out, "b_out": b_out}
        for c in range(NCORES)
    ]
    res = run_bass_kernel_spmd(nc, in_maps, core_ids=list(range(NCORES)))
    out = np.concatenate(
        [_from_blocks_T_host(res.results[c]["o"]) for c in range(NCORES)],
        axis=0)
    return out.astype(np.float32)
